# revision 1
# baseline (speedup 1.0000x reference)
"""Trainium2 Bass kernel for nn_DivTree (moe_routing) — bf16 + fused L3.

Computation (per reference):
    x1 = relu(x0 @ W_shared + b_shared)         # [B, A, H]
    h  = relu(einsum('bah,ahk', x1, W1[route]) + b1[route])
    y  = einsum('bah,ahk', h, W2[route]) + b2[route]   # [B, A, NA]

Strategy: data-parallel over batch across 8 NeuronCores (512 rows/core),
weights replicated, agents grouped by expert (8 distinct experts).
Feature-major layout for L1/L2: contraction on SBUF partitions, weights
stationary, batch as the 512-wide moving free dim. All matmul operands
bf16 (fp32 PSUM accumulation, fp32 output): same PE stream rate as
f32r, but FWL halves LDWEIGHTS and DMA bytes halve.

Trace-driven pipeline fixes (v2/v3b post-mortems):
  * x0 is prefetched 3 agents ahead (xpool bufs=4), and its DMA
    triggers issue from the otherwise-idle GpSimd queue: the Sync
    queue's serial ~610ns-per-trigger execution was delaying
    prefetches behind weight/output DMAs (~300ns PE stall per agent).
  * ALL of L3 (4 contiguous feature-major matmuls, W2 stationary) is
    deferred until after the NEXT agent's L1 matmuls: the h m-tile
    activations complete with ~3.4us of slack, so the PE never waits
    on them (interleaving L3 into L2 left only ~860ns of slack — less
    than the ~900ns activation latency — and each joint also paid
    ~105ns of LDWEIGHTS serialization).
  * Head DMA issue is parallelized across engine queues (x0 on
    GpSimd, shared-trunk weights on Scalar, expert weights on Sync)
    so the first matmul isn't gated on one serial trigger queue.
"""

import numpy as np

P = 128
N_CORES = 8

_cache: dict = {}


def _build(A, D, H, NA, Bl, groups):
    import concourse.mybir as mybir
    import concourse.tile as tile
    from concourse import bacc
    from contextlib import ExitStack

    f32 = mybir.dt.float32
    bf16 = mybir.dt.bfloat16
    Relu = mybir.ActivationFunctionType.Relu
    E = len(groups)
    KD, KH, MH = D // P, H // P, H // P
    NB = Bl  # matmul free dim (batch); Bl=512 fits one PSUM bank
    JB = NB // P  # batch blocks of 128 for L3 (stationary columns)
    assert NB <= 512 and H % P == 0 and D % P == 0 and NA <= P

    agent_list = [(s, a) for s, agents in enumerate(groups) for a in agents]
    NAG = len(agent_list)

    nc = bacc.Bacc()
    x0t = nc.declare_dram_parameter("x0t", [A, D, Bl], bf16, isOutput=False)
    ws = nc.declare_dram_parameter("ws", [D, H], bf16, isOutput=False)
    bs = nc.declare_dram_parameter("bs", [H], f32, isOutput=False)
    w1g = nc.declare_dram_parameter("w1g", [E, H, H], bf16, isOutput=False)
    b1g = nc.declare_dram_parameter("b1g", [E, H], f32, isOutput=False)
    w2g = nc.declare_dram_parameter("w2g", [E, H, NA], bf16, isOutput=False)
    b2r = nc.declare_dram_parameter("b2r", [E, NA, 1], f32, isOutput=False)
    yt = nc.declare_dram_parameter("yt", [A, NA, Bl], f32, isOutput=True)

    with tile.TileContext(nc) as tc, ExitStack() as ctx:
        const = ctx.enter_context(tc.tile_pool(name="const", bufs=1))
        wpool = ctx.enter_context(tc.tile_pool(name="wexp", bufs=2))
        xpool = ctx.enter_context(tc.tile_pool(name="x0", bufs=5))
        x1pool = ctx.enter_context(tc.tile_pool(name="x1", bufs=5))
        hpool = ctx.enter_context(tc.tile_pool(name="h", bufs=5))
        opool = ctx.enter_context(tc.tile_pool(name="out", bufs=3))
        psum = ctx.enter_context(tc.tile_pool(name="ps", bufs=3, space="PSUM"))
        psum2 = ctx.enter_context(tc.tile_pool(name="ps2", bufs=3, space="PSUM"))
        psum3 = ctx.enter_context(tc.tile_pool(name="ps3", bufs=2, space="PSUM"))

        # PE warm-up: the HAM clock gate holds the array at 1.2GHz until it
        # has been busy ~3.4us. Burn dummy matmuls during the initial DMA
        # wait so the real matmuls start at full clock.
        dummy = const.tile([P, 128], bf16)
        nc.gpsimd.memset(dummy[:], 0.0)
        dps = psum.tile([64, 128], f32, tag="ps")
        for i in range(26):
            nc.tensor.matmul(dps[:], dummy[:, :64], dummy[:, :128],
                             start=True, stop=True)

        # the first agent's input and the first shared-weight m-tile gate
        # the first matmul: load them before anything else, in k-subtile
        # pieces so the PE can start early
        ws_r = ws.rearrange("(ks p) h -> p ks h", p=P)
        a0 = agent_list[0][1]
        x0_first = xpool.tile([P, KD, NB], bf16, tag="x0")
        x0_first_r = x0t[a0].rearrange("(ks p) b -> p ks b", p=P)
        wsm = [const.tile([P, KD, P], bf16, tag=f"wsm{ms}", name=f"wsm{ms}")
               for ms in range(MH)]
        # parallel trigger issue: x0 pieces round-robin across the three
        # DMA-capable engine queues so their transfers land on different
        # DMA rings and arrive in parallel (one ring moves a 128KB piece
        # in ~1.1us; serial arrival gated the first agent's L1)
        dma_engines = [nc.gpsimd, nc.sync, nc.scalar]
        # x0_first pieces split gpsimd/scalar so they arrive pairwise in
        # parallel (~1.1us per 128KB piece per ring); scalar's queue
        # drains well before its first ACTIVATE. Shared trunk (wsm, bs)
        # on sync, k0-first, ahead of the group-0 expert weights.
        for ks in range(KD):
            eng = nc.gpsimd if ks < 2 else nc.scalar
            eng.dma_start(x0_first[:, ks, :], x0_first_r[:, ks, :])
            nc.sync.dma_start(wsm[0][:, ks, :], ws_r[:, ks, 0:P])
        for ms in range(1, MH):
            nc.sync.dma_start(wsm[ms][:], ws_r[:, :, ms * P:(ms + 1) * P])
        bs_t = const.tile([P, MH], f32)
        nc.sync.dma_start(bs_t[:], bs.rearrange("(ms p) -> p ms", p=P))

        x0_tiles = {0: x0_first}

        def dma_x0(t, head=False):
            if t >= NAG or t in x0_tiles:
                return
            a = agent_list[t][1]
            x0_t = xpool.tile([P, KD, NB], bf16, tag="x0", name=f"x0_{a}")
            x0_r = x0t[a].rearrange("(ks p) b -> p ks b", p=P)
            if head:
                # split per k-subtile across two engine queues so the
                # pieces ride different DMA rings and arrive in parallel
                # (the head has no prefetch slack to hide a serial tile)
                for ks in range(KD):
                    eng = nc.gpsimd if ks < 2 else nc.scalar
                    eng.dma_start(x0_t[:, ks, :], x0_r[:, ks, :])
            else:
                # steady state is prefetched 3 agents (~21us) ahead: one
                # 512KB transfer (~4.4us on one ring) arrives early even
                # unsplit, and 1 trigger instead of 4 keeps the engine
                # trigger queues (~610ns per trigger) clear
                dma_engines[t % 3].dma_start(x0_t[:], x0_r[:])
            x0_tiles[t] = x0_t

        def load_group_weights(s):
            w1_t = wpool.tile([P, KH, H], bf16, tag="w1", name=f"w1_{s}")
            w1_r = w1g[s].rearrange("(ks p) h -> p ks h", p=P)
            for ks in range(KH):
                nc.sync.dma_start(w1_t[:, ks, :], w1_r[:, ks, :])
            b1_t = wpool.tile([P, MH], f32, tag="b1", name=f"b1_{s}")
            nc.sync.dma_start(b1_t[:], b1g[s].rearrange("(ms p) -> p ms", p=P))
            w2_t = wpool.tile([P, KH, NA], bf16, tag="w2", name=f"w2_{s}")
            nc.sync.dma_start(
                w2_t[:], w2g[s].rearrange("(ks p) n -> p ks n", p=P))
            b2_t = wpool.tile([NA, 1], f32, tag="b2", name=f"b2_{s}")
            nc.sync.dma_start(b2_t[:], b2r[s])
            return (w1_t, b1_t, w2_t, b2_t)

        def emit_l1(a, x0_t):
            x1_t = x1pool.tile([P, MH, NB], bf16, tag="x1", name=f"x1_{a}")
            for ms in range(MH):
                ps1 = psum.tile([P, NB], f32, tag="ps", name=f"ps1_{a}_{ms}")
                for ks in range(KD):
                    nc.tensor.matmul(
                        ps1[:], wsm[ms][:, ks, :], x0_t[:, ks, :],
                        start=(ks == 0), stop=(ks == KD - 1),
                    )
                if ms % 2:
                    nc.vector.tensor_scalar(
                        x1_t[:, ms, :], ps1[:], bs_t[:, ms:ms + 1], 0.0,
                        mybir.AluOpType.add, mybir.AluOpType.max)
                else:
                    nc.scalar.activation(x1_t[:, ms, :], ps1[:], Relu,
                                         bias=bs_t[:, ms:ms + 1])
            return x1_t

        def emit_l2(a, x1_t, wt):
            w1_t, b1_t, w2_t, b2_t = wt
            h_t = hpool.tile([P, MH, NB], bf16, tag="h", name=f"h_{a}")
            for ms in range(MH):
                ps2 = psum2.tile([P, NB], f32, tag="ps2", name=f"ps2_{a}_{ms}")
                for ks in range(KH):
                    nc.tensor.matmul(
                        ps2[:],
                        w1_t[:, ks, ms * P:(ms + 1) * P],
                        x1_t[:, ks, :],
                        start=(ks == 0), stop=(ks == KH - 1),
                    )
                if ms % 2:
                    nc.vector.tensor_scalar(
                        h_t[:, ms, :], ps2[:], b1_t[:, ms:ms + 1], 0.0,
                        mybir.AluOpType.add, mybir.AluOpType.max)
                else:
                    nc.scalar.activation(h_t[:, ms, :], ps2[:], Relu,
                                         bias=b1_t[:, ms:ms + 1])
            return h_t

        def emit_l3_tail(a, h_t, wt):
            w1_t, b1_t, w2_t, b2_t = wt
            ps3 = psum3.tile([P, NB], f32, tag="ps3", name=f"ps3_{a}")
            for k in range(KH):
                nc.tensor.matmul(
                    ps3[:NA, :],
                    w2_t[:, k, :],
                    h_t[:, k, :],
                    start=(k == 0), stop=(k == KH - 1),
                )
            o_t = opool.tile([NA, NB], f32, tag="o", name=f"o_{a}")
            nc.vector.tensor_add(
                o_t[:], ps3[:NA, :],
                b2_t[:NA, 0:1].to_broadcast((NA, NB)),
            )
            # the store trigger is deferred to the NEXT iteration, after
            # its x0 prefetch triggers: on the sync queue this trigger
            # blocks until the DVE add completes, and anything queued
            # behind it (x0 prefetches!) would inherit that wait
            return (a, o_t)

        # two-stage software pipeline over agent PAIRS: each round emits
        #   L1(a), L1(a+1) | L3(a-4), L3(a-3) | L2(a-2), L2(a-1)
        # Fewer PE section boundaries (each boundary exposes an
        # un-prefetched LDWEIGHTS plus a cross-engine semaphore check,
        # ~100-300ns) than the per-agent pipeline.
        pend_l2 = []     # [(a, x1_t, wt)] — L1 done, L2 not yet emitted
        pend_tail = []   # [(a, h_t, wt)] — L2 done, L3 deferred
        pend_store = []  # [(a, o_t)] — output computed, store not queued
        cur_s = -1
        wt = None
        for t, (s, a) in enumerate(agent_list):
            if s != cur_s:
                wt = load_group_weights(s)
                cur_s = s
            if t == 0:
                dma_x0(1, head=True)
                dma_x0(2, head=True)
            dma_x0(t + 3)
            for sa, so in pend_store:
                nc.sync.dma_start(yt[sa], so[:])
            pend_store = []
            x1_t = emit_l1(a, x0_tiles.pop(t))
            pend_l2.append((a, x1_t, wt))
            if t % 2 == 1:
                for args in pend_tail:
                    pend_store.append(emit_l3_tail(*args))
                pend_tail = []
                while len(pend_l2) > 2:
                    pa, px1, pwt = pend_l2.pop(0)
                    h_t = emit_l2(pa, px1, pwt)
                    pend_tail.append((pa, h_t, pwt))
        # drain — both remaining L2s before both L3 tails, so the first
        # tail's activations get a full L2 phase of slack
        for sa, so in pend_store:
            nc.sync.dma_start(yt[sa], so[:])
        for args in pend_tail:
            sa, so = emit_l3_tail(*args)
            nc.sync.dma_start(yt[sa], so[:])
        done_l2 = []
        for pa, px1, pwt in pend_l2:
            done_l2.append((pa, emit_l2(pa, px1, pwt), pwt))
        for pa, h_t, pwt in done_l2:
            sa, so = emit_l3_tail(pa, h_t, pwt)
            nc.sync.dma_start(yt[sa], so[:])

    nc.compile()
    return nc


def kernel(x0, W_shared, b_shared, W1, b1, W2, b2, route,
           _trace=False, _tmpdir=None):
    import ml_dtypes
    from concourse.bass_utils import run_bass_kernel_spmd

    bf16 = ml_dtypes.bfloat16
    x0 = np.asarray(x0, dtype=np.float32)
    W_shared = np.asarray(W_shared, dtype=np.float32)
    b_shared = np.asarray(b_shared, dtype=np.float32)
    W1 = np.asarray(W1, dtype=np.float32)
    b1 = np.asarray(b1, dtype=np.float32)
    W2 = np.asarray(W2, dtype=np.float32)
    b2 = np.asarray(b2, dtype=np.float32)
    route = np.asarray(route)

    B, A, D = x0.shape
    H = W_shared.shape[1]
    NA = W2.shape[2]
    Bl = B // N_CORES
    JB = Bl // P

    experts, inv = np.unique(route, return_inverse=True)
    groups = tuple(tuple(np.where(inv == s)[0].tolist())
                   for s in range(len(experts)))

    key = (B, A, D, H, NA, groups)
    nc = _cache.get(key)
    if nc is None:
        nc = _build(A, D, H, NA, Bl, groups)
        _cache[key] = nc

    # host-side shard + transpose to feature-major, cast to bf16,
    # gather distinct experts
    x0t = np.ascontiguousarray(
        x0.astype(bf16).reshape(N_CORES, Bl, A, D).transpose(0, 2, 3, 1))
    w1g = np.ascontiguousarray(W1[experts].astype(bf16))
    b1g = np.ascontiguousarray(b1[experts])
    w2g = np.ascontiguousarray(W2[experts].astype(bf16))
    b2r = np.ascontiguousarray(b2[experts])[:, :, None]  # [E, NA, 1]
    ws_b = W_shared.astype(bf16)

    in_maps = [
        dict(x0t=x0t[c], ws=ws_b, bs=b_shared,
             w1g=w1g, b1g=b1g, w2g=w2g, b2r=b2r)
        for c in range(N_CORES)
    ]
    # the axon-proxied runtime occasionally reports a transient
    # "device unrecoverable" right after another process released the
    # cores; a short-delay retry recovers it
    import time
    last_err = None
    for attempt in range(3):
        try:
            res = run_bass_kernel_spmd(nc, in_maps,
                                       core_ids=list(range(N_CORES)),
                                       trace=_trace, tmpdir=_tmpdir)
            break
        except Exception as e:  # noqa: BLE001
            last_err = e
            time.sleep(5.0 * (attempt + 1))
    else:
        raise last_err
    kernel.last_exec_time_ns = res.exec_time_ns
    yt = np.stack([res.results[c]["yt"] for c in range(N_CORES)])  # [NC,A,NA,Bl]
    y = np.ascontiguousarray(yt.transpose(0, 3, 1, 2)).reshape(B, A, NA)
    return y



# revision 4
# speedup vs baseline: 1.2453x; 1.2453x over previous
"""Trainium2 Bass kernel for nn_DivTree (moe_routing) — bf16, preloaded
weights, batch-stationary L3.

Computation (per reference):
    x1 = relu(x0 @ W_shared + b_shared)         # [B, A, H]
    h  = relu(einsum('bah,ahk', x1, W1[route]) + b1[route])
    y  = einsum('bah,ahk', h, W2[route]) + b2[route]   # [B, A, NA]

Strategy: data-parallel over batch across 8 NeuronCores (512 rows/core),
weights replicated, agents grouped by expert (8 distinct experts).
Feature-major layout for L1/L2: contraction on SBUF partitions, weights
stationary, batch as the 512-wide moving free dim. All matmul operands
bf16 (fp32 PSUM accumulation, fp32 output).

Measured-on-HW design notes (microbench, this session):
  * A 512-free bf16 matmul takes ~216ns at full clock (213.3 ideal) —
    the kernel is PE-roofline-bound; run-to-run DVFS moves this to
    235-260ns, which dominates measurement noise.
  * Stationary-weight reloads are free (same-stationary vs cycling
    stationary: identical timing) — LDWEIGHTS fully overlaps.
  * L3 as [128k, 32m, 512n] wastes 3/4 of the PE (out partitions 32).
    Batch-stationary form — stationary = h [128k, 128b], moving =
    W2 [128k, 32] — runs at ~27-36ns per matmul: 16 tiny matmuls
    (~500ns) replace 4 big ones (~940ns) per agent, saving ~14us of
    PE time across 32 agents.
  * ALL expert weights (4.5MB bf16) are preloaded into SBUF at the
    head (34KB/partition; budget ~101KB of 208KB): steady state has
    zero weight DMAs, and group transitions cost nothing.
  * Head DMAs are spread 4-wide across the sync/gpsimd/scalar/vector
    trigger queues (each trigger costs ~610ns of queue time): the
    first agent's x0 k-pieces and the first shared-weight m-tile ride
    4 parallel queues so L1(a0) can start ~9.5us instead of ~13us.
  * fp8 DoubleRow doubles PE throughput but e4m3 quantization of even
    ONE layer gives 3.5e-2 Frobenius error vs the 2e-2 gate — dead.

Pipeline (unchanged from baseline v3b): two-stage software pipeline
over agent pairs, L3 of agents t-4,t-3 deferred until after L1 of the
pair t,t+1 so the PE never waits on h activations; output stores are
deferred one iteration so their queue-blocking wait never delays x0
prefetch triggers.
"""

import numpy as np

P = 128
N_CORES = 8
WARMUP = 20

_cache: dict = {}


def _build(A, D, H, NA, Bl, groups):
    import concourse.mybir as mybir
    import concourse.tile as tile
    from concourse import bacc
    from contextlib import ExitStack

    f32 = mybir.dt.float32
    bf16 = mybir.dt.bfloat16
    Relu = mybir.ActivationFunctionType.Relu
    E = len(groups)
    KD, KH, MH = D // P, H // P, H // P
    NB = Bl  # matmul free dim (batch); Bl=512 fits one PSUM bank
    JB = NB // P  # batch blocks of 128 (stationary columns in L3)
    assert NB <= 512 and H % P == 0 and D % P == 0 and NA <= P

    agent_list = [(s, a) for s, agents in enumerate(groups) for a in agents]
    NAG = len(agent_list)

    nc = bacc.Bacc()
    x0t = nc.declare_dram_parameter("x0t", [A, D, Bl], bf16, isOutput=False)
    ws = nc.declare_dram_parameter("ws", [D, H], bf16, isOutput=False)
    bs = nc.declare_dram_parameter("bs", [H], f32, isOutput=False)
    w1g = nc.declare_dram_parameter("w1g", [E, H, H], bf16, isOutput=False)
    b1g = nc.declare_dram_parameter("b1g", [E, H], f32, isOutput=False)
    w2g = nc.declare_dram_parameter("w2g", [E, H, NA], bf16, isOutput=False)
    b2rep = nc.declare_dram_parameter("b2rep", [E, P, NA], f32, isOutput=False)
    yt = nc.declare_dram_parameter("yt", [A, Bl, NA], f32, isOutput=True)

    with tile.TileContext(nc) as tc, ExitStack() as ctx:
        const = ctx.enter_context(tc.tile_pool(name="const", bufs=1))
        xpool = ctx.enter_context(tc.tile_pool(name="x0", bufs=5))
        x1pool = ctx.enter_context(tc.tile_pool(name="x1", bufs=5))
        hpool = ctx.enter_context(tc.tile_pool(name="h", bufs=5))
        opool = ctx.enter_context(tc.tile_pool(name="out", bufs=3))
        psum = ctx.enter_context(tc.tile_pool(name="ps", bufs=3, space="PSUM"))
        psum2 = ctx.enter_context(tc.tile_pool(name="ps2", bufs=3, space="PSUM"))
        psum3 = ctx.enter_context(tc.tile_pool(name="ps3", bufs=2, space="PSUM"))

        # PE warm-up: the HAM clock gate holds the array at low clock until
        # it has been busy a while. Burn dummy matmuls during the initial
        # DMA wait so the real matmuls start at a higher clock.
        dummy = const.tile([P, 128], bf16)
        nc.gpsimd.memset(dummy[:], 0.0)
        dps = psum.tile([64, 128], f32, tag="ps")
        for i in range(WARMUP):
            nc.tensor.matmul(dps[:], dummy[:, :64], dummy[:, :128],
                             start=True, stop=True)

        # ---- head DMA issue, spread across 4 trigger queues ----
        # q[ks] carries (x0[a0] piece ks, wsm0 piece ks) so the first L1
        # chain's operands ride 4 parallel queues.
        ws_r = ws.rearrange("(ks p) h -> p ks h", p=P)
        a0 = agent_list[0][1]
        x0_first = xpool.tile([P, KD, NB], bf16, tag="x0")
        x0_first_r = x0t[a0].rearrange("(ks p) b -> p ks b", p=P)
        wsm = [const.tile([P, KD, P], bf16, tag=f"wsm{ms}", name=f"wsm{ms}")
               for ms in range(MH)]
        # only gpsimd / sync(SP) / scalar(Activation) can trigger DMAs;
        # interleave the critical first-chain pieces across all three
        headq = [nc.sync, nc.gpsimd, nc.scalar, nc.gpsimd]
        for ks in range(KD):
            headq[ks].dma_start(x0_first[:, ks, :], x0_first_r[:, ks, :])
            headq[ks].dma_start(wsm[0][:, ks, :], ws_r[:, ks, 0:P])
        # remaining shared-trunk m-tiles + bias, spread by deadline
        nc.sync.dma_start(wsm[1][:], ws_r[:, :, P:2 * P])
        bs_t = const.tile([P, MH], f32)
        nc.scalar.dma_start(bs_t[:], bs.rearrange("(ms p) -> p ms", p=P))
        nc.scalar.dma_start(wsm[2][:], ws_r[:, :, 2 * P:3 * P])
        nc.gpsimd.dma_start(wsm[3][:], ws_r[:, :, 3 * P:4 * P])

        # preloaded expert weights: all groups resident in SBUF
        w1all = const.tile([P, E, KH, H], bf16, name="w1all")
        w2all = const.tile([P, E, KH, NA], bf16, name="w2all")
        b1all = const.tile([P, E, MH], f32, name="b1all")
        b2all = const.tile([P, E, NA], f32, name="b2all")
        # group 0's W1 is on the first-L2 critical path (~14us)
        nc.scalar.dma_start(
            w1all[:, 0], w1g[0].rearrange("(ks p) h -> p ks h", p=P))
        # second/third agents' x0 (deadlines ~18/25us), single triggers
        x0_tiles = {0: x0_first}

        def dma_x0(t, eng):
            if t >= NAG or t in x0_tiles:
                return
            a = agent_list[t][1]
            x0_t = xpool.tile([P, KD, NB], bf16, tag="x0", name=f"x0_{a}")
            eng.dma_start(x0_t[:], x0t[a].rearrange("(ks p) b -> p ks b", p=P))
            x0_tiles[t] = x0_t

        dma_x0(1, nc.gpsimd)
        dma_x0(2, nc.sync)
        # bias vectors + small weights, then the remaining groups' W1
        # (group s is first needed around t ≈ 11us + 27us*s)
        nc.sync.dma_start(
            b1all[:], b1g.rearrange("e (ms p) -> p e ms", p=P))
        nc.sync.dma_start(
            b2all[:], b2rep.rearrange("e p n -> p e n"))
        nc.sync.dma_start(
            w2all[:], w2g.rearrange("e (ks p) n -> p e ks n", p=P))
        w1q = {1: nc.gpsimd, 2: nc.scalar, 3: nc.sync, 4: nc.scalar,
               5: nc.sync, 6: nc.sync, 7: nc.sync}
        for s in range(1, E):
            w1q[s].dma_start(
                w1all[:, s], w1g[s].rearrange("(ks p) h -> p ks h", p=P))

        # steady-state x0 prefetch: one trigger, rotating queues (sync is
        # reserved for output stores)
        dma_engines = [nc.gpsimd, nc.scalar]

        def emit_l1(a, x0_t):
            x1_t = x1pool.tile([P, MH, NB], bf16, tag="x1", name=f"x1_{a}")
            for ms in range(MH):
                ps1 = psum.tile([P, NB], f32, tag="ps", name=f"ps1_{a}_{ms}")
                for ks in range(KD):
                    nc.tensor.matmul(
                        ps1[:], wsm[ms][:, ks, :], x0_t[:, ks, :],
                        start=(ks == 0), stop=(ks == KD - 1),
                    )
                if ms % 2:
                    nc.vector.tensor_scalar(
                        x1_t[:, ms, :], ps1[:], bs_t[:, ms:ms + 1], 0.0,
                        mybir.AluOpType.add, mybir.AluOpType.max)
                else:
                    nc.scalar.activation(x1_t[:, ms, :], ps1[:], Relu,
                                         bias=bs_t[:, ms:ms + 1])
            return x1_t

        def emit_l2(a, x1_t, s):
            h_t = hpool.tile([P, MH, NB], bf16, tag="h", name=f"h_{a}")
            for ms in range(MH):
                ps2 = psum2.tile([P, NB], f32, tag="ps2", name=f"ps2_{a}_{ms}")
                for ks in range(KH):
                    nc.tensor.matmul(
                        ps2[:],
                        w1all[:, s, ks, ms * P:(ms + 1) * P],
                        x1_t[:, ks, :],
                        start=(ks == 0), stop=(ks == KH - 1),
                    )
                if ms % 2:
                    nc.vector.tensor_scalar(
                        h_t[:, ms, :], ps2[:], b1all[:, s, ms:ms + 1], 0.0,
                        mybir.AluOpType.add, mybir.AluOpType.max)
                else:
                    nc.scalar.activation(h_t[:, ms, :], ps2[:], Relu,
                                         bias=b1all[:, s, ms:ms + 1])
            return h_t

        def emit_l3_tail(a, h_t, s):
            # batch-stationary: stationary = h [128k, 128b-block], moving =
            # W2 [128k, NA]; out [128b, NA] accumulated over KH k-tiles.
            ps3 = psum3.tile([P, JB, NA], f32, tag="ps3", name=f"ps3_{a}")
            for jb in range(JB):
                for ks in range(KH):
                    nc.tensor.matmul(
                        ps3[:, jb, :],
                        h_t[:, ks, jb * P:(jb + 1) * P],
                        w2all[:, s, ks, :],
                        start=(ks == 0), stop=(ks == KH - 1),
                    )
            o_t = opool.tile([P, JB, NA], f32, tag="o", name=f"o_{a}")
            nc.vector.tensor_add(
                o_t[:], ps3[:],
                b2all[:, s:s + 1, :].to_broadcast((P, JB, NA)),
            )
            # store trigger deferred to the NEXT iteration: on the sync
            # queue it blocks until the DVE add completes, and anything
            # queued behind it would inherit that wait
            return (a, o_t)

        def store(a, o_t):
            nc.sync.dma_start(
                yt[a].rearrange("(j p) n -> p j n", p=P), o_t[:])

        # two-stage software pipeline over agent PAIRS: each round emits
        #   L1(a), L1(a+1) | L3(a-4), L3(a-3) | L2(a-2), L2(a-1)
        pend_l2 = []     # [(a, x1_t, s)] — L1 done, L2 not yet emitted
        pend_tail = []   # [(a, h_t, s)] — L2 done, L3 deferred
        pend_store = []  # [(a, o_t)] — output computed, store not queued
        for t, (s, a) in enumerate(agent_list):
            dma_x0(t + 3, dma_engines[t % 2])
            for sa, so in pend_store:
                store(sa, so)
            pend_store = []
            x1_t = emit_l1(a, x0_tiles.pop(t))
            pend_l2.append((a, x1_t, s))
            if t % 2 == 1:
                for args in pend_tail:
                    pend_store.append(emit_l3_tail(*args))
                pend_tail = []
                while len(pend_l2) > 2:
                    pa, px1, ps_ = pend_l2.pop(0)
                    h_t = emit_l2(pa, px1, ps_)
                    pend_tail.append((pa, h_t, ps_))
        # drain — both remaining L2s before both L3 tails, so the first
        # tail's activations get a full L2 phase of slack
        for sa, so in pend_store:
            store(sa, so)
        for args in pend_tail:
            store(*emit_l3_tail(*args))
        done_l2 = []
        for pa, px1, ps_ in pend_l2:
            done_l2.append((pa, emit_l2(pa, px1, ps_), ps_))
        for pa, h_t, ps_ in done_l2:
            store(*emit_l3_tail(pa, h_t, ps_))

    nc.compile()
    return nc


def kernel(x0, W_shared, b_shared, W1, b1, W2, b2, route,
           _trace=False, _tmpdir=None):
    import ml_dtypes
    from concourse.bass_utils import run_bass_kernel_spmd

    bf16 = ml_dtypes.bfloat16
    x0 = np.asarray(x0, dtype=np.float32)
    W_shared = np.asarray(W_shared, dtype=np.float32)
    b_shared = np.asarray(b_shared, dtype=np.float32)
    W1 = np.asarray(W1, dtype=np.float32)
    b1 = np.asarray(b1, dtype=np.float32)
    W2 = np.asarray(W2, dtype=np.float32)
    b2 = np.asarray(b2, dtype=np.float32)
    route = np.asarray(route)

    B, A, D = x0.shape
    H = W_shared.shape[1]
    NA = W2.shape[2]
    Bl = B // N_CORES

    experts, inv = np.unique(route, return_inverse=True)
    groups = tuple(tuple(np.where(inv == s)[0].tolist())
                   for s in range(len(experts)))
    E = len(experts)

    key = (B, A, D, H, NA, groups)
    nc = _cache.get(key)
    if nc is None:
        nc = _build(A, D, H, NA, Bl, groups)
        _cache[key] = nc

    # host-side shard + transpose to feature-major, cast to bf16,
    # gather distinct experts
    x0t = np.ascontiguousarray(
        x0.astype(bf16).reshape(N_CORES, Bl, A, D).transpose(0, 2, 3, 1))
    w1g = np.ascontiguousarray(W1[experts].astype(bf16))
    b1g = np.ascontiguousarray(b1[experts])
    w2g = np.ascontiguousarray(W2[experts].astype(bf16))
    b2rep = np.ascontiguousarray(
        np.broadcast_to(b2[experts][:, None, :], (E, P, NA)))
    ws_b = W_shared.astype(bf16)

    in_maps = [
        dict(x0t=x0t[c], ws=ws_b, bs=b_shared,
             w1g=w1g, b1g=b1g, w2g=w2g, b2rep=b2rep)
        for c in range(N_CORES)
    ]
    # the axon-proxied runtime occasionally reports a transient
    # "device unrecoverable" right after another process released the
    # cores; a short-delay retry recovers it
    import time
    last_err = None
    for attempt in range(3):
        try:
            res = run_bass_kernel_spmd(nc, in_maps,
                                       core_ids=list(range(N_CORES)),
                                       trace=_trace, tmpdir=_tmpdir)
            break
        except Exception as e:  # noqa: BLE001
            last_err = e
            time.sleep(5.0 * (attempt + 1))
    else:
        raise last_err
    kernel.last_exec_time_ns = res.exec_time_ns
    yt = np.stack([res.results[c]["yt"] for c in range(N_CORES)])  # [NC,A,Bl,NA]
    y = np.ascontiguousarray(yt.transpose(0, 2, 1, 3)).reshape(B, A, NA)
    return y


# revision 5
# speedup vs baseline: 1.2638x; 1.0149x over previous
"""Trainium2 Bass kernel for nn_DivTree (moe_routing) — bf16, preloaded
weights, batch-stationary L3, partition-major DRAM layouts.

Computation (per reference):
    x1 = relu(x0 @ W_shared + b_shared)         # [B, A, H]
    h  = relu(einsum('bah,ahk', x1, W1[route]) + b1[route])
    y  = einsum('bah,ahk', h, W2[route]) + b2[route]   # [B, A, NA]

Strategy: data-parallel over batch across 8 NeuronCores (512 rows/core),
weights replicated, agents grouped by expert (8 distinct experts).
Feature-major layout for L1/L2: contraction on SBUF partitions, weights
stationary, batch as the 512-wide moving free dim. All matmul operands
bf16 (fp32 PSUM accumulation, fp32 output).

Measured-on-HW design notes (microbench + trace, this session):
  * A 512-free bf16 matmul takes ~216ns at full clock (213.3 ideal) —
    PE-roofline-bound; run-to-run DVFS moves this to 235-260ns.
  * LDWEIGHTS fully overlaps the previous matmul (97ns slices in its
    shadow); stationary reloads are free.
  * L3 batch-stationary — stationary = h [128k, 128b], moving = W2
    [128k, 32] — runs at ~26ns/matmul: 16 tiny matmuls (~0.5us)
    replace 4 big ones (~0.94us) per agent: −14us of PE time.
  * ALL expert weights (4.5MB bf16) are preloaded into SBUF (34KB of
    the 208KB partition budget): steady state has zero weight DMAs.
  * DMA triggers cost ~8ns PER DESCRIPTOR on the issuing queue
    (DIRECT2D slices): a strided rearrange like "e (ms p) -> p e ms"
    is a descriptor bomb (4096 descs = 32us of queue time). Every
    DRAM tensor is therefore pre-transposed on the host so each
    transfer is one contiguous chunk per partition (128 descriptors,
    ~1us of queue time), including the output stores.
  * fp8 DoubleRow doubles PE throughput but e4m3 quantization of even
    ONE layer gives 3.5e-2 Frobenius error vs the 2e-2 gate — dead.

Pipeline: two-stage software pipeline over agent pairs, L3 of agents
t-4,t-3 deferred until after L1 of the pair t,t+1 so the PE never waits
on h activations; output stores are deferred one iteration so their
queue-blocking wait never delays x0 prefetch triggers. Head DMAs are
spread across the three DMA-capable queues (sync/gpsimd/scalar) with
the first L1 chain's operands split piecewise across all three.
"""

import numpy as np

P = 128
N_CORES = 8
WARMUP = 20

_cache: dict = {}


def _build(A, D, H, NA, Bl, groups):
    import concourse.mybir as mybir
    import concourse.tile as tile
    from concourse import bacc
    from contextlib import ExitStack

    f32 = mybir.dt.float32
    bf16 = mybir.dt.bfloat16
    Relu = mybir.ActivationFunctionType.Relu
    E = len(groups)
    KD, KH, MH = D // P, H // P, H // P
    NB = Bl  # matmul free dim (batch); Bl=512 fits one PSUM bank
    JB = NB // P  # batch blocks of 128 (stationary columns in L3)
    assert NB <= 512 and H % P == 0 and D % P == 0 and NA <= P

    agent_list = [(s, a) for s, agents in enumerate(groups) for a in agents]
    NAG = len(agent_list)

    nc = bacc.Bacc()
    # partition-major layouts: every DMA moves one contiguous chunk per
    # partition (see module docstring)
    x0p = nc.declare_dram_parameter("x0p", [A, P, KD, Bl], bf16,
                                    isOutput=False)
    wsp = nc.declare_dram_parameter("wsp", [P, MH, KD, P], bf16,
                                    isOutput=False)
    bsp = nc.declare_dram_parameter("bsp", [P, MH], f32, isOutput=False)
    w1p = nc.declare_dram_parameter("w1p", [E, P, KH, H], bf16,
                                    isOutput=False)
    b1p = nc.declare_dram_parameter("b1p", [P, E, MH], f32, isOutput=False)
    w2p = nc.declare_dram_parameter("w2p", [P, E, KH, NA], bf16,
                                    isOutput=False)
    b2p = nc.declare_dram_parameter("b2p", [P, E, NA], f32, isOutput=False)
    yt = nc.declare_dram_parameter("yt", [A, P, JB, NA], f32, isOutput=True)

    with tile.TileContext(nc) as tc, ExitStack() as ctx:
        const = ctx.enter_context(tc.tile_pool(name="const", bufs=1))
        xpool = ctx.enter_context(tc.tile_pool(name="x0", bufs=5))
        x1pool = ctx.enter_context(tc.tile_pool(name="x1", bufs=5))
        hpool = ctx.enter_context(tc.tile_pool(name="h", bufs=5))
        opool = ctx.enter_context(tc.tile_pool(name="out", bufs=3))
        psum = ctx.enter_context(tc.tile_pool(name="ps", bufs=3, space="PSUM"))
        psum2 = ctx.enter_context(tc.tile_pool(name="ps2", bufs=3, space="PSUM"))
        psum3 = ctx.enter_context(tc.tile_pool(name="ps3", bufs=2, space="PSUM"))

        # PE warm-up: the HAM clock gate holds the array at low clock until
        # it has been busy a while; burn dummy matmuls during the initial
        # DMA wait so real matmuls start at a higher clock.
        dummy = const.tile([P, 128], bf16)
        nc.gpsimd.memset(dummy[:], 0.0)
        dps = psum.tile([64, 128], f32, tag="ps")
        for i in range(WARMUP):
            nc.tensor.matmul(dps[:], dummy[:, :64], dummy[:, :128],
                             start=True, stop=True)

        # ---- head DMA issue, spread across the 3 trigger queues ----
        a0 = agent_list[0][1]
        x0_first = xpool.tile([P, KD, NB], bf16, tag="x0")
        wsm = [const.tile([P, KD, P], bf16, tag=f"wsm{ms}", name=f"wsm{ms}")
               for ms in range(MH)]
        headq = [nc.sync, nc.gpsimd, nc.scalar, nc.gpsimd]
        for ks in range(KD):
            headq[ks].dma_start(x0_first[:, ks, :], x0p[a0][:, ks, :])
            headq[ks].dma_start(wsm[0][:, ks, :], wsp[:, 0, ks, :])
        nc.sync.dma_start(wsm[1][:], wsp[:, 1])
        bs_t = const.tile([P, MH], f32)
        nc.gpsimd.dma_start(bs_t[:], bsp[:, :])
        nc.scalar.dma_start(wsm[2][:], wsp[:, 2])
        nc.gpsimd.dma_start(wsm[3][:], wsp[:, 3])

        # preloaded expert weights: all groups resident in SBUF
        w1all = const.tile([P, E, KH, H], bf16, name="w1all")
        w2all = const.tile([P, E, KH, NA], bf16, name="w2all")
        b1all = const.tile([P, E, MH], f32, name="b1all")
        b2all = const.tile([P, E, NA], f32, name="b2all")
        # group 0's W1 is on the first-L2 critical path (~14us)
        nc.scalar.dma_start(w1all[:, 0], w1p[0])

        x0_tiles = {0: x0_first}

        def dma_x0(t, eng, eng2=None):
            if t >= NAG or t in x0_tiles:
                return
            a = agent_list[t][1]
            x0_t = xpool.tile([P, KD, NB], bf16, tag="x0", name=f"x0_{a}")
            if eng2 is not None:  # head: split halves across two queues
                eng.dma_start(x0_t[:, :KD // 2], x0p[a][:, :KD // 2])
                eng2.dma_start(x0_t[:, KD // 2:], x0p[a][:, KD // 2:])
            else:
                eng.dma_start(x0_t[:], x0p[a])
            x0_tiles[t] = x0_t

        dma_x0(1, nc.gpsimd, nc.scalar)
        dma_x0(2, nc.sync, nc.gpsimd)
        # bias vectors + small weights, then the remaining groups' W1
        # (group s is first needed around t ≈ 11us + 27us*s)
        nc.sync.dma_start(b1all[:], b1p[:, :])
        nc.sync.dma_start(b2all[:], b2p[:, :])
        nc.sync.dma_start(w2all[:], w2p[:, :])
        w1q = {1: nc.gpsimd, 2: nc.scalar, 3: nc.sync, 4: nc.scalar,
               5: nc.sync, 6: nc.sync, 7: nc.sync}
        for s in range(1, E):
            w1q[s].dma_start(w1all[:, s], w1p[s])

        # steady-state x0 prefetch: one trigger, rotating queues (sync is
        # reserved for output stores)
        dma_engines = [nc.gpsimd, nc.scalar]

        def emit_l1(a, x0_t):
            x1_t = x1pool.tile([P, MH, NB], bf16, tag="x1", name=f"x1_{a}")
            for ms in range(MH):
                ps1 = psum.tile([P, NB], f32, tag="ps", name=f"ps1_{a}_{ms}")
                for ks in range(KD):
                    nc.tensor.matmul(
                        ps1[:], wsm[ms][:, ks, :], x0_t[:, ks, :],
                        start=(ks == 0), stop=(ks == KD - 1),
                    )
                if ms % 2:
                    nc.vector.tensor_scalar(
                        x1_t[:, ms, :], ps1[:], bs_t[:, ms:ms + 1], 0.0,
                        mybir.AluOpType.add, mybir.AluOpType.max)
                else:
                    nc.scalar.activation(x1_t[:, ms, :], ps1[:], Relu,
                                         bias=bs_t[:, ms:ms + 1])
            return x1_t

        def emit_l2(a, x1_t, s):
            h_t = hpool.tile([P, MH, NB], bf16, tag="h", name=f"h_{a}")
            for ms in range(MH):
                ps2 = psum2.tile([P, NB], f32, tag="ps2", name=f"ps2_{a}_{ms}")
                for ks in range(KH):
                    nc.tensor.matmul(
                        ps2[:],
                        w1all[:, s, ks, ms * P:(ms + 1) * P],
                        x1_t[:, ks, :],
                        start=(ks == 0), stop=(ks == KH - 1),
                    )
                if ms % 2:
                    nc.vector.tensor_scalar(
                        h_t[:, ms, :], ps2[:], b1all[:, s, ms:ms + 1], 0.0,
                        mybir.AluOpType.add, mybir.AluOpType.max)
                else:
                    nc.scalar.activation(h_t[:, ms, :], ps2[:], Relu,
                                         bias=b1all[:, s, ms:ms + 1])
            return h_t

        def emit_l3_tail(a, h_t, s):
            # batch-stationary: stationary = h [128k, 128b-block], moving =
            # W2 [128k, NA]; out [128b, NA] accumulated over KH k-tiles.
            ps3 = psum3.tile([P, JB, NA], f32, tag="ps3", name=f"ps3_{a}")
            for jb in range(JB):
                for ks in range(KH):
                    nc.tensor.matmul(
                        ps3[:, jb, :],
                        h_t[:, ks, jb * P:(jb + 1) * P],
                        w2all[:, s, ks, :],
                        start=(ks == 0), stop=(ks == KH - 1),
                    )
            o_t = opool.tile([P, JB, NA], f32, tag="o", name=f"o_{a}")
            nc.vector.tensor_add(
                o_t[:], ps3[:],
                b2all[:, s:s + 1, :].to_broadcast((P, JB, NA)),
            )
            # store trigger deferred to the NEXT iteration: on the sync
            # queue it blocks until the DVE add completes, and anything
            # queued behind it would inherit that wait
            return (a, o_t)

        def store(a, o_t):
            nc.sync.dma_start(yt[a], o_t[:])

        # two-stage software pipeline over agent PAIRS: each round emits
        #   L1(a), L1(a+1) | L3(a-4), L3(a-3) | L2(a-2), L2(a-1)
        pend_l2 = []     # [(a, x1_t, s)] — L1 done, L2 not yet emitted
        pend_tail = []   # [(a, h_t, s)] — L2 done, L3 deferred
        pend_store = []  # [(a, o_t)] — output computed, store not queued
        for t, (s, a) in enumerate(agent_list):
            dma_x0(t + 3, dma_engines[t % 2])
            for sa, so in pend_store:
                store(sa, so)
            pend_store = []
            x1_t = emit_l1(a, x0_tiles.pop(t))
            pend_l2.append((a, x1_t, s))
            if t % 2 == 1:
                for args in pend_tail:
                    pend_store.append(emit_l3_tail(*args))
                pend_tail = []
                while len(pend_l2) > 2:
                    pa, px1, ps_ = pend_l2.pop(0)
                    h_t = emit_l2(pa, px1, ps_)
                    pend_tail.append((pa, h_t, ps_))
        # drain — both remaining L2s before both L3 tails, so the first
        # tail's activations get a full L2 phase of slack
        for sa, so in pend_store:
            store(sa, so)
        for args in pend_tail:
            store(*emit_l3_tail(*args))
        done_l2 = []
        for pa, px1, ps_ in pend_l2:
            done_l2.append((pa, emit_l2(pa, px1, ps_), ps_))
        for pa, h_t, ps_ in done_l2:
            store(*emit_l3_tail(pa, h_t, ps_))

    nc.compile()
    return nc


def kernel(x0, W_shared, b_shared, W1, b1, W2, b2, route,
           _trace=False, _tmpdir=None):
    import ml_dtypes
    from concourse.bass_utils import run_bass_kernel_spmd

    bf16 = ml_dtypes.bfloat16
    x0 = np.asarray(x0, dtype=np.float32)
    W_shared = np.asarray(W_shared, dtype=np.float32)
    b_shared = np.asarray(b_shared, dtype=np.float32)
    W1 = np.asarray(W1, dtype=np.float32)
    b1 = np.asarray(b1, dtype=np.float32)
    W2 = np.asarray(W2, dtype=np.float32)
    b2 = np.asarray(b2, dtype=np.float32)
    route = np.asarray(route)

    B, A, D = x0.shape
    H = W_shared.shape[1]
    NA = W2.shape[2]
    Bl = B // N_CORES
    KD, KH, MH = D // P, H // P, H // P
    JB = Bl // P

    experts, inv = np.unique(route, return_inverse=True)
    groups = tuple(tuple(np.where(inv == s)[0].tolist())
                   for s in range(len(experts)))
    E = len(experts)

    key = (B, A, D, H, NA, groups)
    nc = _cache.get(key)
    if nc is None:
        nc = _build(A, D, H, NA, Bl, groups)
        _cache[key] = nc

    # host-side shard + transpose to partition-major bf16 layouts
    # x0p[c][a, p, ks, b] = x0[c*Bl + b, a, ks*P + p]
    x0p = np.ascontiguousarray(
        x0.astype(bf16).reshape(N_CORES, Bl, A, KD, P)
        .transpose(0, 2, 4, 3, 1))
    # wsp[p, ms, ks, q] = Ws[ks*P + p, ms*P + q]
    wsp = np.ascontiguousarray(
        W_shared.astype(bf16).reshape(KD, P, MH, P).transpose(1, 2, 0, 3))
    bsp = np.ascontiguousarray(b_shared.reshape(MH, P).T)
    # w1p[s, p, ks, h] = W1[experts[s], ks*P + p, h]
    w1p = np.ascontiguousarray(
        W1[experts].astype(bf16).reshape(E, KH, P, H).transpose(0, 2, 1, 3))
    # b1p[p, s, ms] = b1[experts[s], ms*P + p]
    b1p = np.ascontiguousarray(
        b1[experts].reshape(E, MH, P).transpose(2, 0, 1))
    # w2p[p, s, ks, n] = W2[experts[s], ks*P + p, n]
    w2p = np.ascontiguousarray(
        W2[experts].astype(bf16).reshape(E, KH, P, NA).transpose(2, 0, 1, 3))
    b2p = np.ascontiguousarray(
        np.broadcast_to(b2[experts][None, :, :], (P, E, NA)))

    in_maps = [
        dict(x0p=x0p[c], wsp=wsp, bsp=bsp,
             w1p=w1p, b1p=b1p, w2p=w2p, b2p=b2p)
        for c in range(N_CORES)
    ]
    # the axon-proxied runtime occasionally reports a transient
    # "device unrecoverable" right after another process released the
    # cores; a short-delay retry recovers it
    import time
    last_err = None
    for attempt in range(3):
        try:
            res = run_bass_kernel_spmd(nc, in_maps,
                                       core_ids=list(range(N_CORES)),
                                       trace=_trace, tmpdir=_tmpdir)
            break
        except Exception as e:  # noqa: BLE001
            last_err = e
            time.sleep(5.0 * (attempt + 1))
    else:
        raise last_err
    kernel.last_exec_time_ns = res.exec_time_ns
    # yt[c][a, p, j, n] = y[c*Bl + j*P + p, a, n]
    yt = np.stack([res.results[c]["yt"] for c in range(N_CORES)])
    y = np.ascontiguousarray(
        yt.transpose(0, 3, 2, 1, 4)).reshape(B, A, NA)
    return y


# revision 7
# speedup vs baseline: 1.2709x; 1.0056x over previous
"""Trainium2 Bass kernel for nn_DivTree (moe_routing) — bf16, preloaded
weights, batch-stationary L3, partition-major DRAM layouts.

Computation (per reference):
    x1 = relu(x0 @ W_shared + b_shared)         # [B, A, H]
    h  = relu(einsum('bah,ahk', x1, W1[route]) + b1[route])
    y  = einsum('bah,ahk', h, W2[route]) + b2[route]   # [B, A, NA]

Strategy: data-parallel over batch across 8 NeuronCores (512 rows/core),
weights replicated, agents grouped by expert (8 distinct experts).
Feature-major layout for L1/L2: contraction on SBUF partitions, weights
stationary, batch as the 512-wide moving free dim. All matmul operands
bf16 (fp32 PSUM accumulation, fp32 output).

Measured-on-HW design notes (microbench + trace, this session):
  * A 512-free bf16 matmul takes ~216ns at full clock (213.3 ideal) —
    PE-roofline-bound; run-to-run DVFS moves this to 235-260ns.
  * LDWEIGHTS fully overlaps the previous matmul (97ns slices in its
    shadow); stationary reloads are free.
  * L3 batch-stationary — stationary = h [128k, 128b], moving = W2
    [128k, 32] — runs at ~26ns/matmul: 16 tiny matmuls (~0.5us)
    replace 4 big ones (~0.94us) per agent: −14us of PE time.
  * ALL expert weights (4.5MB bf16) are preloaded into SBUF (34KB of
    the 208KB partition budget): steady state has zero weight DMAs.
  * DMA triggers cost ~8ns PER DESCRIPTOR on the issuing queue
    (DIRECT2D slices): a strided rearrange like "e (ms p) -> p e ms"
    is a descriptor bomb (4096 descs = 32us of queue time). Every
    DRAM tensor is therefore pre-transposed on the host so each
    transfer is one contiguous chunk per partition (128 descriptors,
    ~1us of queue time), including the output stores.
  * fp8 DoubleRow doubles PE throughput but e4m3 quantization of even
    ONE layer gives 3.5e-2 Frobenius error vs the 2e-2 gate — dead.

Pipeline: two-stage software pipeline over agent pairs, L3 of agents
t-4,t-3 deferred until after L1 of the pair t,t+1 so the PE never waits
on h activations; output stores are deferred one iteration so their
queue-blocking wait never delays x0 prefetch triggers. Head DMAs are
spread across the three DMA-capable queues (sync/gpsimd/scalar) with
the first L1 chain's operands split piecewise across all three.
"""

import numpy as np

P = 128
N_CORES = 8
WARMUP = 16

_cache: dict = {}


def _build(A, D, H, NA, Bl, groups):
    import concourse.mybir as mybir
    import concourse.tile as tile
    from concourse import bacc
    from contextlib import ExitStack

    f32 = mybir.dt.float32
    bf16 = mybir.dt.bfloat16
    Relu = mybir.ActivationFunctionType.Relu
    E = len(groups)
    KD, KH, MH = D // P, H // P, H // P
    NB = Bl  # matmul free dim (batch); Bl=512 fits one PSUM bank
    JB = NB // P  # batch blocks of 128 (stationary columns in L3)
    assert NB <= 512 and H % P == 0 and D % P == 0 and NA <= P

    agent_list = [(s, a) for s, agents in enumerate(groups) for a in agents]
    NAG = len(agent_list)

    nc = bacc.Bacc()
    # partition-major layouts: every DMA moves one contiguous chunk per
    # partition (see module docstring)
    x0p = nc.declare_dram_parameter("x0p", [A, P, KD, Bl], bf16,
                                    isOutput=False)
    wsp = nc.declare_dram_parameter("wsp", [P, MH, KD, P], bf16,
                                    isOutput=False)
    bsp = nc.declare_dram_parameter("bsp", [P, MH], f32, isOutput=False)
    w1p = nc.declare_dram_parameter("w1p", [E, P, KH, H], bf16,
                                    isOutput=False)
    b1p = nc.declare_dram_parameter("b1p", [P, E, MH], f32, isOutput=False)
    w2p = nc.declare_dram_parameter("w2p", [P, E, KH, NA], bf16,
                                    isOutput=False)
    b2p = nc.declare_dram_parameter("b2p", [P, E, NA], f32, isOutput=False)
    yt = nc.declare_dram_parameter("yt", [A, P, JB, NA], f32, isOutput=True)

    with tile.TileContext(nc) as tc, ExitStack() as ctx:
        const = ctx.enter_context(tc.tile_pool(name="const", bufs=1))
        xpool = ctx.enter_context(tc.tile_pool(name="x0", bufs=5))
        x1pool = ctx.enter_context(tc.tile_pool(name="x1", bufs=5))
        hpool = ctx.enter_context(tc.tile_pool(name="h", bufs=5))
        opool = ctx.enter_context(tc.tile_pool(name="out", bufs=3))
        psum = ctx.enter_context(tc.tile_pool(name="ps", bufs=3, space="PSUM"))
        psum2 = ctx.enter_context(tc.tile_pool(name="ps2", bufs=3, space="PSUM"))
        psum3 = ctx.enter_context(tc.tile_pool(name="ps3", bufs=2, space="PSUM"))

        # PE warm-up: the HAM clock gate holds the array at low clock until
        # it has been busy a while; burn dummy matmuls during the initial
        # DMA wait so real matmuls start at a higher clock.
        dummy = const.tile([P, 128], bf16)
        nc.gpsimd.memset(dummy[:], 0.0)
        dps = psum.tile([64, 128], f32, tag="ps")
        for i in range(WARMUP):
            nc.tensor.matmul(dps[:], dummy[:, :64], dummy[:, :128],
                             start=True, stop=True)

        # ---- head DMA issue, spread across the 3 trigger queues ----
        # each trigger costs ~610ns of queue time, so the first L1 chain's
        # operands (x0[a0] k-pieces, wsm0) take the FIRST two slots of
        # every queue; everything else follows by deadline
        a0 = agent_list[0][1]
        x0_first = xpool.tile([P, KD, NB], bf16, tag="x0")
        wsm = [const.tile([P, KD, P], bf16, tag=f"wsm{ms}", name=f"wsm{ms}")
               for ms in range(MH)]
        headq = [nc.sync, nc.gpsimd, nc.scalar, nc.sync]
        for ks in range(KD):
            headq[ks].dma_start(x0_first[:, ks, :], x0p[a0][:, ks, :])
        nc.gpsimd.dma_start(wsm[0][:], wsp[:, 0])
        nc.scalar.dma_start(wsm[1][:], wsp[:, 1])
        nc.sync.dma_start(wsm[2][:], wsp[:, 2])
        nc.gpsimd.dma_start(wsm[3][:], wsp[:, 3])
        bs_t = const.tile([P, MH], f32)
        nc.scalar.dma_start(bs_t[:], bsp[:, :])

        # preloaded expert weights: all groups resident in SBUF
        w1all = const.tile([P, E, KH, H], bf16, name="w1all")
        w2all = const.tile([P, E, KH, NA], bf16, name="w2all")
        b1all = const.tile([P, E, MH], f32, name="b1all")
        b2all = const.tile([P, E, NA], f32, name="b2all")

        x0_tiles = {0: x0_first}

        def dma_x0(t, eng, eng2=None):
            if t >= NAG or t in x0_tiles:
                return
            a = agent_list[t][1]
            x0_t = xpool.tile([P, KD, NB], bf16, tag="x0", name=f"x0_{a}")
            if eng2 is not None:  # head: split halves across two queues
                eng.dma_start(x0_t[:, :KD // 2], x0p[a][:, :KD // 2])
                eng2.dma_start(x0_t[:, KD // 2:], x0p[a][:, KD // 2:])
            else:
                eng.dma_start(x0_t[:], x0p[a])
            x0_tiles[t] = x0_t

        dma_x0(1, nc.gpsimd, nc.scalar)
        # group 0's W1 is on the first-L2 critical path (~14us)
        nc.scalar.dma_start(w1all[:, 0], w1p[0])
        dma_x0(2, nc.sync, nc.gpsimd)
        # bias vectors + small weights, then the remaining groups' W1
        # (group s is first needed around t ≈ 11us + 27us*s)
        nc.sync.dma_start(b1all[:], b1p[:, :])
        nc.sync.dma_start(b2all[:], b2p[:, :])
        nc.sync.dma_start(w2all[:], w2p[:, :])
        w1q = {1: nc.gpsimd, 2: nc.scalar, 3: nc.sync, 4: nc.scalar,
               5: nc.sync, 6: nc.sync, 7: nc.sync}
        for s in range(1, E):
            w1q[s].dma_start(w1all[:, s], w1p[s])

        # steady-state x0 prefetch: one trigger, rotating queues (sync is
        # reserved for output stores)
        dma_engines = [nc.gpsimd, nc.scalar]

        def emit_l1(a, x0_t):
            x1_t = x1pool.tile([P, MH, NB], bf16, tag="x1", name=f"x1_{a}")
            for ms in range(MH):
                ps1 = psum.tile([P, NB], f32, tag="ps", name=f"ps1_{a}_{ms}")
                for ks in range(KD):
                    nc.tensor.matmul(
                        ps1[:], wsm[ms][:, ks, :], x0_t[:, ks, :],
                        start=(ks == 0), stop=(ks == KD - 1),
                    )
                if ms % 2:
                    nc.vector.tensor_scalar(
                        x1_t[:, ms, :], ps1[:], bs_t[:, ms:ms + 1], 0.0,
                        mybir.AluOpType.add, mybir.AluOpType.max)
                else:
                    nc.scalar.activation(x1_t[:, ms, :], ps1[:], Relu,
                                         bias=bs_t[:, ms:ms + 1])
            return x1_t

        def emit_l2(a, x1_t, s):
            h_t = hpool.tile([P, MH, NB], bf16, tag="h", name=f"h_{a}")
            for ms in range(MH):
                ps2 = psum2.tile([P, NB], f32, tag="ps2", name=f"ps2_{a}_{ms}")
                for ks in range(KH):
                    nc.tensor.matmul(
                        ps2[:],
                        w1all[:, s, ks, ms * P:(ms + 1) * P],
                        x1_t[:, ks, :],
                        start=(ks == 0), stop=(ks == KH - 1),
                    )
                if ms % 2:
                    nc.vector.tensor_scalar(
                        h_t[:, ms, :], ps2[:], b1all[:, s, ms:ms + 1], 0.0,
                        mybir.AluOpType.add, mybir.AluOpType.max)
                else:
                    nc.scalar.activation(h_t[:, ms, :], ps2[:], Relu,
                                         bias=b1all[:, s, ms:ms + 1])
            return h_t

        def emit_l3_tail(a, h_t, s):
            # batch-stationary: stationary = h [128k, 128b-block], moving =
            # W2 [128k, NA]; out [128b, NA] accumulated over KH k-tiles.
            ps3 = psum3.tile([P, JB, NA], f32, tag="ps3", name=f"ps3_{a}")
            for jb in range(JB):
                for ks in range(KH):
                    nc.tensor.matmul(
                        ps3[:, jb, :],
                        h_t[:, ks, jb * P:(jb + 1) * P],
                        w2all[:, s, ks, :],
                        start=(ks == 0), stop=(ks == KH - 1),
                    )
            o_t = opool.tile([P, JB, NA], f32, tag="o", name=f"o_{a}")
            nc.vector.tensor_add(
                o_t[:], ps3[:],
                b2all[:, s:s + 1, :].to_broadcast((P, JB, NA)),
            )
            # store trigger deferred to the NEXT iteration: on the sync
            # queue it blocks until the DVE add completes, and anything
            # queued behind it would inherit that wait
            return (a, o_t)

        def store(a, o_t):
            nc.sync.dma_start(yt[a], o_t[:])

        # two-stage software pipeline over agent PAIRS: each round emits
        #   L1(a), L1(a+1) | L3(a-4), L3(a-3) | L2(a-2), L2(a-1)
        pend_l2 = []     # [(a, x1_t, s)] — L1 done, L2 not yet emitted
        pend_tail = []   # [(a, h_t, s)] — L2 done, L3 deferred
        pend_store = []  # [(a, o_t)] — output computed, store not queued
        for t, (s, a) in enumerate(agent_list):
            dma_x0(t + 3, dma_engines[t % 2])
            for sa, so in pend_store:
                store(sa, so)
            pend_store = []
            x1_t = emit_l1(a, x0_tiles.pop(t))
            pend_l2.append((a, x1_t, s))
            if t % 2 == 1:
                for args in pend_tail:
                    pend_store.append(emit_l3_tail(*args))
                pend_tail = []
                while len(pend_l2) > 2:
                    pa, px1, ps_ = pend_l2.pop(0)
                    h_t = emit_l2(pa, px1, ps_)
                    pend_tail.append((pa, h_t, ps_))
        # drain — both remaining L2s before both L3 tails, so the first
        # tail's activations get a full L2 phase of slack
        for sa, so in pend_store:
            store(sa, so)
        for args in pend_tail:
            store(*emit_l3_tail(*args))
        done_l2 = []
        for pa, px1, ps_ in pend_l2:
            done_l2.append((pa, emit_l2(pa, px1, ps_), ps_))
        for pa, h_t, ps_ in done_l2:
            store(*emit_l3_tail(pa, h_t, ps_))

    nc.compile()
    return nc


def kernel(x0, W_shared, b_shared, W1, b1, W2, b2, route,
           _trace=False, _tmpdir=None):
    import ml_dtypes
    from concourse.bass_utils import run_bass_kernel_spmd

    bf16 = ml_dtypes.bfloat16
    x0 = np.asarray(x0, dtype=np.float32)
    W_shared = np.asarray(W_shared, dtype=np.float32)
    b_shared = np.asarray(b_shared, dtype=np.float32)
    W1 = np.asarray(W1, dtype=np.float32)
    b1 = np.asarray(b1, dtype=np.float32)
    W2 = np.asarray(W2, dtype=np.float32)
    b2 = np.asarray(b2, dtype=np.float32)
    route = np.asarray(route)

    B, A, D = x0.shape
    H = W_shared.shape[1]
    NA = W2.shape[2]
    Bl = B // N_CORES
    KD, KH, MH = D // P, H // P, H // P
    JB = Bl // P

    experts, inv = np.unique(route, return_inverse=True)
    groups = tuple(tuple(np.where(inv == s)[0].tolist())
                   for s in range(len(experts)))
    E = len(experts)

    key = (B, A, D, H, NA, groups)
    nc = _cache.get(key)
    if nc is None:
        nc = _build(A, D, H, NA, Bl, groups)
        _cache[key] = nc

    # host-side shard + transpose to partition-major bf16 layouts
    # x0p[c][a, p, ks, b] = x0[c*Bl + b, a, ks*P + p]
    x0p = np.ascontiguousarray(
        x0.astype(bf16).reshape(N_CORES, Bl, A, KD, P)
        .transpose(0, 2, 4, 3, 1))
    # wsp[p, ms, ks, q] = Ws[ks*P + p, ms*P + q]
    wsp = np.ascontiguousarray(
        W_shared.astype(bf16).reshape(KD, P, MH, P).transpose(1, 2, 0, 3))
    bsp = np.ascontiguousarray(b_shared.reshape(MH, P).T)
    # w1p[s, p, ks, h] = W1[experts[s], ks*P + p, h]
    w1p = np.ascontiguousarray(
        W1[experts].astype(bf16).reshape(E, KH, P, H).transpose(0, 2, 1, 3))
    # b1p[p, s, ms] = b1[experts[s], ms*P + p]
    b1p = np.ascontiguousarray(
        b1[experts].reshape(E, MH, P).transpose(2, 0, 1))
    # w2p[p, s, ks, n] = W2[experts[s], ks*P + p, n]
    w2p = np.ascontiguousarray(
        W2[experts].astype(bf16).reshape(E, KH, P, NA).transpose(2, 0, 1, 3))
    b2p = np.ascontiguousarray(
        np.broadcast_to(b2[experts][None, :, :], (P, E, NA)))

    in_maps = [
        dict(x0p=x0p[c], wsp=wsp, bsp=bsp,
             w1p=w1p, b1p=b1p, w2p=w2p, b2p=b2p)
        for c in range(N_CORES)
    ]
    # the axon-proxied runtime occasionally reports a transient
    # "device unrecoverable" right after another process released the
    # cores; a short-delay retry recovers it
    import time
    last_err = None
    for attempt in range(3):
        try:
            res = run_bass_kernel_spmd(nc, in_maps,
                                       core_ids=list(range(N_CORES)),
                                       trace=_trace, tmpdir=_tmpdir)
            break
        except Exception as e:  # noqa: BLE001
            last_err = e
            time.sleep(5.0 * (attempt + 1))
    else:
        raise last_err
    kernel.last_exec_time_ns = res.exec_time_ns
    # yt[c][a, p, j, n] = y[c*Bl + j*P + p, a, n]
    yt = np.stack([res.results[c]["yt"] for c in range(N_CORES)])
    y = np.ascontiguousarray(
        yt.transpose(0, 3, 2, 1, 4)).reshape(B, A, NA)
    return y


# revision 11
# speedup vs baseline: 1.2948x; 1.0188x over previous
"""Trainium2 Bass kernel for nn_DivTree (moe_routing) — bf16, preloaded
weights, batch-stationary L3, partition-major DRAM layouts.

Computation (per reference):
    x1 = relu(x0 @ W_shared + b_shared)         # [B, A, H]
    h  = relu(einsum('bah,ahk', x1, W1[route]) + b1[route])
    y  = einsum('bah,ahk', h, W2[route]) + b2[route]   # [B, A, NA]

Strategy: data-parallel over batch across 8 NeuronCores (512 rows/core),
weights replicated, agents grouped by expert (8 distinct experts).
Feature-major layout for L1/L2: contraction on SBUF partitions, weights
stationary, batch as the 512-wide moving free dim. All matmul operands
bf16 (fp32 PSUM accumulation, fp32 output).

Measured-on-HW design notes (microbench + trace, this session):
  * A 512-free bf16 matmul takes ~216ns at full clock (213.3 ideal) —
    PE-roofline-bound; run-to-run DVFS moves this to 235-260ns.
  * LDWEIGHTS fully overlaps the previous matmul (97ns slices in its
    shadow); stationary reloads are free.
  * L3 batch-stationary — stationary = h [128k, 128b], moving = W2
    [128k, 32] — runs at ~26ns/matmul: 16 tiny matmuls (~0.5us)
    replace 4 big ones (~0.94us) per agent: −14us of PE time.
  * ALL expert weights (4.5MB bf16) are preloaded into SBUF (34KB of
    the 208KB partition budget): steady state has zero weight DMAs.
  * DMA triggers cost ~8ns PER DESCRIPTOR on the issuing queue
    (DIRECT2D slices): a strided rearrange like "e (ms p) -> p e ms"
    is a descriptor bomb (4096 descs = 32us of queue time). Every
    DRAM tensor is therefore pre-transposed on the host so each
    transfer is one contiguous chunk per partition (128 descriptors,
    ~1us of queue time), including the output stores.
  * fp8 DoubleRow doubles PE throughput but e4m3 quantization of even
    ONE layer gives 3.5e-2 Frobenius error vs the 2e-2 gate — dead.

Pipeline: two-stage software pipeline over agent pairs, L3 of agents
t-4,t-3 deferred until after L1 of the pair t,t+1 so the PE never waits
on h activations; output stores are deferred one iteration so their
queue-blocking wait never delays x0 prefetch triggers. Head DMAs are
spread across the three DMA-capable queues (sync/gpsimd/scalar) with
the first L1 chain's operands split piecewise across all three.
"""

import numpy as np

P = 128
N_CORES = 8
WARMUP = 18

_cache: dict = {}


def _build(A, D, H, NA, Bl, groups):
    import concourse.mybir as mybir
    import concourse.tile as tile
    from concourse import bacc
    from contextlib import ExitStack

    f32 = mybir.dt.float32
    bf16 = mybir.dt.bfloat16
    Relu = mybir.ActivationFunctionType.Relu
    E = len(groups)
    KD, KH, MH = D // P, H // P, H // P
    NB = Bl  # matmul free dim (batch); Bl=512 fits one PSUM bank
    JB = NB // P  # batch blocks of 128 (stationary columns in L3)
    assert NB <= 512 and H % P == 0 and D % P == 0 and NA <= P

    agent_list = [(s, a) for s, agents in enumerate(groups) for a in agents]
    NAG = len(agent_list)

    nc = bacc.Bacc()
    # partition-major layouts: every DMA moves one contiguous chunk per
    # partition (see module docstring)
    x0p = nc.declare_dram_parameter("x0p", [A, P, KD, Bl], bf16,
                                    isOutput=False)
    wsp = nc.declare_dram_parameter("wsp", [P, MH, KD, P], bf16,
                                    isOutput=False)
    bsp = nc.declare_dram_parameter("bsp", [P, MH], f32, isOutput=False)
    w1p = nc.declare_dram_parameter("w1p", [E, P, KH, H], bf16,
                                    isOutput=False)
    b1p = nc.declare_dram_parameter("b1p", [P, E, MH], f32, isOutput=False)
    w2p = nc.declare_dram_parameter("w2p", [P, E, KH, NA], bf16,
                                    isOutput=False)
    b2p = nc.declare_dram_parameter("b2p", [P, E, NA], f32, isOutput=False)
    yt = nc.declare_dram_parameter("yt", [A, P, JB, NA], f32, isOutput=True)

    with tile.TileContext(nc) as tc, ExitStack() as ctx:
        const = ctx.enter_context(tc.tile_pool(name="const", bufs=1))
        xpool = ctx.enter_context(tc.tile_pool(name="x0", bufs=5))
        x1pool = ctx.enter_context(tc.tile_pool(name="x1", bufs=5))
        hpool = ctx.enter_context(tc.tile_pool(name="h", bufs=5))
        opool = ctx.enter_context(tc.tile_pool(name="out", bufs=3))
        psum = ctx.enter_context(tc.tile_pool(name="ps", bufs=3, space="PSUM"))
        psum2 = ctx.enter_context(tc.tile_pool(name="ps2", bufs=3, space="PSUM"))
        psum3 = ctx.enter_context(tc.tile_pool(name="ps3", bufs=2, space="PSUM"))

        # PE warm-up: the HAM clock gate holds the array at low clock until
        # it has been busy a while; burn dummy matmuls during the initial
        # DMA wait so real matmuls start at a higher clock.
        dummy = const.tile([P, 128], bf16)
        nc.gpsimd.memset(dummy[:], 0.0)
        dps = psum.tile([64, 128], f32, tag="ps")
        for i in range(WARMUP):
            nc.tensor.matmul(dps[:], dummy[:, :64], dummy[:, :128],
                             start=True, stop=True)

        # ---- head DMA issue, spread across the 3 trigger queues ----
        # each trigger costs ~610ns of queue time, so the first L1 chain's
        # operands (x0[a0] k-pieces, wsm0) take the FIRST two slots of
        # every queue; everything else follows by deadline
        a0 = agent_list[0][1]
        x0_first = xpool.tile([P, KD, NB], bf16, tag="x0")
        wsm = [const.tile([P, KD, P], bf16, tag=f"wsm{ms}", name=f"wsm{ms}")
               for ms in range(MH)]
        # queue layout (ks pieces of x0[a0]/wsm0 interleaved so every ms
        # chain's k-operands land in issue order):
        #   sync:   x0k0 wsm0k0 x0k3 wsm0k3 wsm3 b1all b2all w2all
        #   gpsimd: x0k1 wsm0k1 wsm1 x0a1-lo x0a2-lo w1g0-lo
        #   scalar: x0k2 wsm0k2 wsm2 bs x0a1-hi x0a2-hi w1g0-hi
        headq = [nc.sync, nc.gpsimd, nc.scalar, nc.sync]
        for ks in range(KD):
            headq[ks].dma_start(x0_first[:, ks, :], x0p[a0][:, ks, :])
            headq[ks].dma_start(wsm[0][:, ks, :], wsp[:, 0, ks, :])
        nc.gpsimd.dma_start(wsm[1][:], wsp[:, 1])
        nc.scalar.dma_start(wsm[2][:], wsp[:, 2])
        nc.sync.dma_start(wsm[3][:], wsp[:, 3])
        bs_t = const.tile([P, MH], f32)
        nc.scalar.dma_start(bs_t[:], bsp[:, :])

        # preloaded expert weights: all groups resident in SBUF
        w1all = const.tile([P, E, KH, H], bf16, name="w1all")
        w2all = const.tile([P, E, KH, NA], bf16, name="w2all")
        b1all = const.tile([P, E, MH], f32, name="b1all")
        b2all = const.tile([P, E, NA], f32, name="b2all")

        x0_tiles = {0: x0_first}

        def dma_x0(t, eng, eng2=None):
            if t >= NAG or t in x0_tiles:
                return
            a = agent_list[t][1]
            x0_t = xpool.tile([P, KD, NB], bf16, tag="x0", name=f"x0_{a}")
            if eng2 is not None:  # head: split halves across two queues
                eng.dma_start(x0_t[:, :KD // 2], x0p[a][:, :KD // 2])
                eng2.dma_start(x0_t[:, KD // 2:], x0p[a][:, KD // 2:])
            else:
                eng.dma_start(x0_t[:], x0p[a])
            x0_tiles[t] = x0_t

        dma_x0(1, nc.gpsimd, nc.scalar)
        dma_x0(2, nc.gpsimd, nc.scalar)
        # group 0's W1 is on the first-L2 critical path (~22us); halves so
        # both rings pull it in parallel
        nc.gpsimd.dma_start(w1all[:, 0, :KH // 2], w1p[0][:, :KH // 2])
        nc.scalar.dma_start(w1all[:, 0, KH // 2:], w1p[0][:, KH // 2:])
        nc.sync.dma_start(b1all[:], b1p[:, :])
        nc.sync.dma_start(b2all[:], b2p[:, :])
        nc.sync.dma_start(w2all[:], w2p[:, :])
        # W1 for groups 1..E-1 (4MB total) is staggered into the agent
        # loop — one group per pair — so the head's x0 transfers don't
        # fight it for DMA-ring bandwidth; group s is first needed around
        # 27us*s with the trigger fired ~20us*s, transfer ~5us.
        w1_pending = list(range(1, E))

        # steady-state x0 prefetch: one trigger, rotating queues (sync is
        # reserved for output stores)
        dma_engines = [nc.gpsimd, nc.scalar]

        def emit_l1(a, x0_t):
            x1_t = x1pool.tile([P, MH, NB], bf16, tag="x1", name=f"x1_{a}")
            for ms in range(MH):
                ps1 = psum.tile([P, NB], f32, tag="ps", name=f"ps1_{a}_{ms}")
                for ks in range(KD):
                    nc.tensor.matmul(
                        ps1[:], wsm[ms][:, ks, :], x0_t[:, ks, :],
                        start=(ks == 0), stop=(ks == KD - 1),
                    )
                if ms % 2:
                    nc.vector.tensor_scalar(
                        x1_t[:, ms, :], ps1[:], bs_t[:, ms:ms + 1], 0.0,
                        mybir.AluOpType.add, mybir.AluOpType.max)
                else:
                    nc.scalar.activation(x1_t[:, ms, :], ps1[:], Relu,
                                         bias=bs_t[:, ms:ms + 1])
            return x1_t

        def emit_l2(a, x1_t, s):
            h_t = hpool.tile([P, MH, NB], bf16, tag="h", name=f"h_{a}")
            for ms in range(MH):
                ps2 = psum2.tile([P, NB], f32, tag="ps2", name=f"ps2_{a}_{ms}")
                for ks in range(KH):
                    nc.tensor.matmul(
                        ps2[:],
                        w1all[:, s, ks, ms * P:(ms + 1) * P],
                        x1_t[:, ks, :],
                        start=(ks == 0), stop=(ks == KH - 1),
                    )
                if ms % 2:
                    nc.vector.tensor_scalar(
                        h_t[:, ms, :], ps2[:], b1all[:, s, ms:ms + 1], 0.0,
                        mybir.AluOpType.add, mybir.AluOpType.max)
                else:
                    nc.scalar.activation(h_t[:, ms, :], ps2[:], Relu,
                                         bias=b1all[:, s, ms:ms + 1])
            return h_t

        def emit_l3_tail(a, h_t, s):
            # batch-stationary: stationary = h [128k, 128b-block], moving =
            # W2 [128k, NA]; out [128b, NA] accumulated over KH k-tiles.
            ps3 = psum3.tile([P, JB, NA], f32, tag="ps3", name=f"ps3_{a}")
            for jb in range(JB):
                for ks in range(KH):
                    nc.tensor.matmul(
                        ps3[:, jb, :],
                        h_t[:, ks, jb * P:(jb + 1) * P],
                        w2all[:, s, ks, :],
                        start=(ks == 0), stop=(ks == KH - 1),
                    )
            o_t = opool.tile([P, JB, NA], f32, tag="o", name=f"o_{a}")
            nc.vector.tensor_add(
                o_t[:], ps3[:],
                b2all[:, s:s + 1, :].to_broadcast((P, JB, NA)),
            )
            # store trigger deferred to the NEXT iteration: on the sync
            # queue it blocks until the DVE add completes, and anything
            # queued behind it would inherit that wait
            return (a, o_t)

        def store(a, o_t):
            nc.sync.dma_start(yt[a], o_t[:])

        # two-stage software pipeline over agent PAIRS: each round emits
        #   L1(a), L1(a+1) | L3(a-4), L3(a-3) | L2(a-2), L2(a-1)
        pend_l2 = []     # [(a, x1_t, s)] — L1 done, L2 not yet emitted
        pend_tail = []   # [(a, h_t, s)] — L2 done, L3 deferred
        pend_store = []  # [(a, o_t)] — output computed, store not queued
        for t, (s, a) in enumerate(agent_list):
            dma_x0(t + 3, dma_engines[t % 2])
            if t % 2 == 0 and w1_pending:
                sw = w1_pending.pop(0)
                nc.sync.dma_start(w1all[:, sw], w1p[sw])
            for sa, so in pend_store:
                store(sa, so)
            pend_store = []
            x1_t = emit_l1(a, x0_tiles.pop(t))
            pend_l2.append((a, x1_t, s))
            if t % 2 == 1:
                for args in pend_tail:
                    pend_store.append(emit_l3_tail(*args))
                pend_tail = []
                while len(pend_l2) > 2:
                    pa, px1, ps_ = pend_l2.pop(0)
                    h_t = emit_l2(pa, px1, ps_)
                    pend_tail.append((pa, h_t, ps_))
        # drain — both remaining L2s before both L3 tails, so the first
        # tail's activations get a full L2 phase of slack
        for sa, so in pend_store:
            store(sa, so)
        for args in pend_tail:
            store(*emit_l3_tail(*args))
        done_l2 = []
        for pa, px1, ps_ in pend_l2:
            done_l2.append((pa, emit_l2(pa, px1, ps_), ps_))
        for pa, h_t, ps_ in done_l2:
            store(*emit_l3_tail(pa, h_t, ps_))

    nc.compile()
    return nc


def kernel(x0, W_shared, b_shared, W1, b1, W2, b2, route,
           _trace=False, _tmpdir=None):
    import ml_dtypes
    from concourse.bass_utils import run_bass_kernel_spmd

    bf16 = ml_dtypes.bfloat16
    x0 = np.asarray(x0, dtype=np.float32)
    W_shared = np.asarray(W_shared, dtype=np.float32)
    b_shared = np.asarray(b_shared, dtype=np.float32)
    W1 = np.asarray(W1, dtype=np.float32)
    b1 = np.asarray(b1, dtype=np.float32)
    W2 = np.asarray(W2, dtype=np.float32)
    b2 = np.asarray(b2, dtype=np.float32)
    route = np.asarray(route)

    B, A, D = x0.shape
    H = W_shared.shape[1]
    NA = W2.shape[2]
    Bl = B // N_CORES
    KD, KH, MH = D // P, H // P, H // P
    JB = Bl // P

    experts, inv = np.unique(route, return_inverse=True)
    groups = tuple(tuple(np.where(inv == s)[0].tolist())
                   for s in range(len(experts)))
    E = len(experts)

    key = (B, A, D, H, NA, groups)
    nc = _cache.get(key)
    if nc is None:
        nc = _build(A, D, H, NA, Bl, groups)
        _cache[key] = nc

    # host-side shard + transpose to partition-major bf16 layouts
    # x0p[c][a, p, ks, b] = x0[c*Bl + b, a, ks*P + p]
    x0p = np.ascontiguousarray(
        x0.astype(bf16).reshape(N_CORES, Bl, A, KD, P)
        .transpose(0, 2, 4, 3, 1))
    # wsp[p, ms, ks, q] = Ws[ks*P + p, ms*P + q]
    wsp = np.ascontiguousarray(
        W_shared.astype(bf16).reshape(KD, P, MH, P).transpose(1, 2, 0, 3))
    bsp = np.ascontiguousarray(b_shared.reshape(MH, P).T)
    # w1p[s, p, ks, h] = W1[experts[s], ks*P + p, h]
    w1p = np.ascontiguousarray(
        W1[experts].astype(bf16).reshape(E, KH, P, H).transpose(0, 2, 1, 3))
    # b1p[p, s, ms] = b1[experts[s], ms*P + p]
    b1p = np.ascontiguousarray(
        b1[experts].reshape(E, MH, P).transpose(2, 0, 1))
    # w2p[p, s, ks, n] = W2[experts[s], ks*P + p, n]
    w2p = np.ascontiguousarray(
        W2[experts].astype(bf16).reshape(E, KH, P, NA).transpose(2, 0, 1, 3))
    b2p = np.ascontiguousarray(
        np.broadcast_to(b2[experts][None, :, :], (P, E, NA)))

    in_maps = [
        dict(x0p=x0p[c], wsp=wsp, bsp=bsp,
             w1p=w1p, b1p=b1p, w2p=w2p, b2p=b2p)
        for c in range(N_CORES)
    ]
    # the axon-proxied runtime occasionally reports a transient
    # "device unrecoverable" right after another process released the
    # cores; a short-delay retry recovers it
    import time
    last_err = None
    for attempt in range(3):
        try:
            res = run_bass_kernel_spmd(nc, in_maps,
                                       core_ids=list(range(N_CORES)),
                                       trace=_trace, tmpdir=_tmpdir)
            break
        except Exception as e:  # noqa: BLE001
            last_err = e
            time.sleep(5.0 * (attempt + 1))
    else:
        raise last_err
    kernel.last_exec_time_ns = res.exec_time_ns
    # yt[c][a, p, j, n] = y[c*Bl + j*P + p, a, n]
    yt = np.stack([res.results[c]["yt"] for c in range(N_CORES)])
    y = np.ascontiguousarray(
        yt.transpose(0, 3, 2, 1, 4)).reshape(B, A, NA)
    return y
